# revision 37
# baseline (speedup 1.0000x reference)
"""Trainium2 Bass kernel for single-head causal attention.

Problem: x:[4,2048,768], Wq/Wk/Wv:[768,768] (torch-Linear layout, y = x @ W.T),
out = causal_softmax(q k^T / sqrt(768)) @ v, all float32.

Sharding (8 NeuronCores, no collectives):
  - core pair (2b, 2b+1) handles batch b.
  - per batch, the 16 query tiles of 128 rows are split between the pair as
    {0,3,4,7,8,11,12,15} and {1,2,5,6,9,10,13,14}. Sorted by causal length
    those are {1,4,5,8,9,12,13,16} and {2,3,6,7,10,11,14,15} key-tiles, so
    both sides fit the same static per-slot key budget {2,4,...,16}: the one
    SPMD graph processes 8 query tiles whose key ranges are padded by at most
    one 128-tile and the pad/diagonal is handled by a host-provided additive
    mask over the last two key blocks of every slot.

Math restructuring vs the naive graph:
  - scores = (x Wq^T)(x Wk^T)^T = x (Wq^T Wk) x^T.  The host precomputes
    M = Wq^T @ Wk once, the device projects only the query rows (z = x_q M)
    and uses raw x^T (already needed as a DMA input) as the key-side operand.
    This deletes the whole K projection, which was computed redundantly by
    both cores of a pair (the single biggest PE-time item).
  - out = (probs @ x) @ Wv^T (the "V-trick"): V is never projected for the
    full sequence; the per-query Y = probs @ x is projected instead.
  - value path (probs @ x and Y @ Wv^T) runs in fp8 e4m3 with DoubleRow
    perf mode (2 contraction tiles per instruction) for slots >= 1; slot 0
    (query tiles 0/1, few-key rows, where softmax weight noise does not
    average out) stays bf16.  exp is computed as exp(s/sqrt(d) - 3) on fp8
    slots so probs fit e4m3's +/-240 range; the shift cancels in the
    softmax normalization.
"""

import math
import os
import sys

import numpy as np

if not any(os.path.isdir(os.path.join(p, "concourse")) for p in sys.path):
    sys.path.insert(0, "/opt/trn_rl_repo")

import concourse.bass as bass  # noqa: E402
import concourse.mybir as mybir  # noqa: E402
from concourse import bacc, tile  # noqa: E402
from concourse.bass_utils import run_bass_kernel_spmd  # noqa: E402
from concourse.masks import make_identity  # noqa: E402

import ml_dtypes  # noqa: E402

B, S, D = 4, 2048, 768
P = 128
NT = S // P          # 16 key tiles per batch
DC = D // P          # 6 contraction chunks
NSLOT = 8            # query tiles per core
QROWS = NSLOT * P    # 1024 query rows per core
N_CORES = 8
SCALE = 1.0 / math.sqrt(D)
C_SHIFT = 3.0        # exp bias on fp8 slots (cancels in normalization)

SIDE_A = [0, 3, 4, 7, 8, 11, 12, 15]   # causal lengths 1,4,5,8,9,12,13,16
SIDE_B = [1, 2, 5, 6, 9, 10, 13, 14]   # causal lengths 2,3,6,7,10,11,14,15
CAP = [2, 4, 6, 8, 10, 12, 14, 16]     # static key tiles per slot (>= real)

BF16 = ml_dtypes.bfloat16
F8E4 = ml_dtypes.float8_e4m3

PV_FP8 = True        # probs @ x in fp8 DoubleRow for slots >= 1
VT_FP8 = False       # fp8 DR loses to bf16 here (tiny MMs, unhidden LDW)

_NC = None


def build():
    """Build + compile the single SPMD graph run by all 8 cores."""
    f32 = mybir.dt.float32
    bf16 = mybir.dt.bfloat16
    f8e4 = mybir.dt.float8e4
    DR = mybir.MatmulPerfMode.DoubleRow

    nc = bacc.Bacc("TRN2", target_bir_lowering=False, debug=False,
                   num_devices=N_CORES)

    # inputs come pre-packed as [P, chunk, width] (host layout transform)
    xq_d = nc.dram_tensor("xqT", [P, 2, DC, 512], bf16,
                          kind="ExternalInput").ap()
    m_d = nc.dram_tensor("mT", [P, 3, DC, 256], bf16,
                         kind="ExternalInput").ap()
    xkv_d = nc.dram_tensor("xkvT", [P, 2, DC, S // 2], bf16,
                           kind="ExternalInput").ap()
    xv8_d = nc.dram_tensor("xv8", [P, NT, D], f8e4, kind="ExternalInput").ap()
    xv16_d = nc.dram_tensor("xv16", [P, NT, D], bf16,
                            kind="ExternalInput").ap()
    wv8_d = nc.dram_tensor("wv8", [P, DC, D], f8e4, kind="ExternalInput").ap()
    wv16_d = nc.dram_tensor("wv16", [P, DC, D], bf16,
                            kind="ExternalInput").ap()
    mask_d = nc.dram_tensor("mask", [P, NSLOT, 256], bf16,
                            kind="ExternalInput").ap()
    f16 = mybir.dt.float16
    out_d = nc.dram_tensor("out", [QROWS, D], f16,
                           kind="ExternalOutput").ap()

    with tile.TileContext(nc) as tc:
        with (
            tc.tile_pool(name="const", bufs=1) as const,
            tc.tile_pool(name="probs", bufs=6) as probs_pool,
            tc.tile_pool(name="lsums", bufs=5) as lsum_pool,
            tc.tile_pool(name="pt", bufs=8) as pt_pool,
            tc.tile_pool(name="osb", bufs=2) as osb_pool,
            tc.tile_pool(name="small", bufs=2) as small,
            tc.tile_pool(name="ps_s", bufs=2, space="PSUM") as ps_s,
            tc.tile_pool(name="ps_tr", bufs=2, space="PSUM") as ps_tr,
            tc.tile_pool(name="ps_o", bufs=2, space="PSUM") as ps_o,
        ):
            HC = DC // 2
            m_p = [const.tile([P, DC, 256], bf16, tag=f"mp{i}",
                              name=f"mp{i}") for i in range(3)]
            xq_g = [const.tile([P, DC, 512], bf16, tag=f"xqg{g}",
                               name=f"xqg{g}") for g in range(2)]
            xkv_k = [const.tile([P, DC, S // 2], bf16, tag=f"xkvk{h}",
                                name=f"xkvk{h}") for h in range(2)]
            wv16_h = [const.tile([P, HC, D], bf16, tag=f"wv16h{h}",
                                 name=f"wv16h{h}") for h in range(2)]
            wv8_sb = (const.tile([P, DC, D], f8e4, tag="wv8",
                                 name="wv8_sb")
                      if VT_FP8 else None)
            xv8_h = [const.tile([P, NT // 2, D], f8e4, tag=f"xv8h{h}",
                                name=f"xv8h{h}") for h in range(2)]
            nv16 = 2 if PV_FP8 else NT
            xv16_sb = const.tile([P, nv16, D], bf16, tag="xv16")
            mask_sb = const.tile([P, NSLOT, 256], bf16, tag="mask")
            ident = const.tile([P, P], bf16, tag="ident")
            cbias = const.tile([P, 1], f32, tag="cbias")
            nc.gpsimd.memset(cbias[:, :], -C_SHIFT)
            zt_sb = const.tile([P, DC, QROWS], bf16, tag="zt")

            def wv16_c(dc):
                return wv16_h[dc // HC][:, dc % HC, :]

            # Input DMAs in WAVES: concurrent transfers fair-share the DMA
            # ring, so everything finishes late together.  The sync queue is
            # in-order, so one gated dma_start (1-element vector copy from
            # the previous wave's last piece creates the dep) holds back the
            # whole next wave, letting each wave run at full bandwidth.
            # wave 1: first Zproj group's operands (m0 + xq group 0)
            for c in range(0, DC, 2):
                nc.sync.dma_start(out=m_p[0][:, c:c + 2, :],
                                  in_=m_d[:, 0, c:c + 2, :])
            for dc in range(DC):
                nc.sync.dma_start(out=xq_g[0][:, dc, :],
                                  in_=xq_d[:, 0, dc, :])
            # wave 2: rest of Zproj (m1, m2, xq group 1)
            nc.vector.tensor_copy(m_p[1][:, 0, 0:1], xq_g[0][:, 5, 0:1])
            for i in range(1, 3):
                for c in range(0, DC, 2):
                    nc.sync.dma_start(out=m_p[i][:, c:c + 2, :],
                                      in_=m_d[:, i, c:c + 2, :])
            for dc in range(DC):
                nc.sync.dma_start(out=xq_g[1][:, dc, :],
                                  in_=xq_d[:, 1, dc, :])
            # wave 3: first score slots' keys + mask
            nc.vector.tensor_copy(xkv_k[0][:, 0, 0:1], xq_g[1][:, 5, 0:1])
            nc.sync.dma_start(out=xkv_k[0][:, :, :], in_=xkv_d[:, 0, :, :])
            nc.sync.dma_start(out=mask_sb[:, :, :], in_=mask_d[:, :, :])

            # HAM warm-up: keep the PE busy while wave 1 streams in so the
            # real matmuls run at 2.4GHz from the start.  The dummy operand
            # needs only a gpsimd memset (make_identity's affine_select
            # would delay the first warm-up matmul).
            dummy = const.tile([P, P], bf16, tag="dummy")
            nc.gpsimd.memset(dummy[:, :], 0.0)
            warm = ps_tr.tile([P, P], f32, tag="tr", name="warm")
            for _ in range(16):
                nc.tensor.matmul(warm[:, :], dummy[:, :], dummy[:, :],
                                 start=True, stop=True)
            make_identity(nc, ident[:, :])

            # ---- zT[e,q] projection: z = x_q @ (Wq^T Wk), group-major
            def zproj_group(g):
                for oc in range(DC):
                    ps = ps_s.tile([P, 512], f32, tag="mm512",
                                   name=f"zps{g}_{oc}")
                    for dc in range(DC):
                        nc.tensor.matmul(
                            ps[:, :],
                            m_p[oc // 2][:, dc,
                                         (oc % 2) * P:(oc % 2 + 1) * P],
                            xq_g[g][:, dc, :],
                            start=(dc == 0), stop=(dc == DC - 1))
                    nc.scalar.copy(zt_sb[:, oc, g * 512:(g + 1) * 512],
                                   ps[:, :])

            for g in range(QROWS // 512):
                zproj_group(g)
                if g == 0:
                    # later inputs gated behind Zproj progress so the DMA
                    # ring's fair-share doesn't starve the critical path
                    gate = zt_sb[:, 0, 0:1]
                    nc.vector.tensor_copy(xv16_sb[:, 0, 0:1], gate)
                    nc.sync.dma_start(out=xv16_sb[:, :, :],
                                      in_=xv16_d[:, 0:nv16, :])
                    for h in range(2):
                        nc.vector.tensor_copy(wv16_h[h][:, 0, 0:1], gate)
                        nc.sync.dma_start(out=wv16_h[h][:, :, :],
                                          in_=wv16_d[:, h * HC:(h + 1) * HC,
                                                     :])
                else:
                    gate = zt_sb[:, 5, 0:1]
                    nc.vector.tensor_copy(xv8_h[0][:, 0, 0:1], gate)
                    nc.sync.dma_start(out=xv8_h[0][:, :, :],
                                      in_=xv8_d[:, 0:NT // 2, :])
                    nc.vector.tensor_copy(xkv_k[1][:, 0, 0:1], gate)
                    nc.sync.dma_start(out=xkv_k[1][:, :, :],
                                      in_=xkv_d[:, 1, :, :])
                    nc.vector.tensor_copy(xv8_h[1][:, 0, 0:1], gate)
                    nc.sync.dma_start(out=xv8_h[1][:, :, :],
                                      in_=xv8_d[:, NT // 2:NT, :])
                    if VT_FP8:
                        nc.sync.dma_start(out=wv8_sb[:, :, :],
                                          in_=wv8_d[:, :, :])

            slot_bufs = {}
            slot_y = {}
            slot_y_ps = {}

            def emit_scores(s):
                L = CAP[s]
                nk = L * P
                nkg = (nk + 511) // 512
                fp8 = PV_FP8 and s > 0
                probs = probs_pool.tile([P, S], bf16, tag="probs",
                                        name=f"probs{s}")
                lsum = lsum_pool.tile([P, 4], f32, tag="lsum",
                                      name=f"lsum{s}")
                slot_bufs[s] = (probs, lsum)
                for g in range(nkg):
                    kw = min(512, nk - g * 512)
                    ps = ps_s.tile([P, 512], f32, tag="mm512",
                                   name=f"sps{s}_{g}")
                    xk = xkv_k[g // 2]
                    ko = (g % 2) * 512
                    for dc in range(DC):
                        nc.tensor.matmul(
                            ps[:, :kw],
                            zt_sb[:, dc, s * P:(s + 1) * P],
                            xk[:, dc, ko:ko + kw],
                            start=(dc == 0), stop=(dc == DC - 1))
                    mstart = (L - 2) * P   # masked region: last two blocks
                    if g * 512 <= mstart < g * 512 + kw:
                        off = mstart - g * 512
                        nc.vector.tensor_add(ps[:, off:off + 256],
                                             ps[:, off:off + 256],
                                             mask_sb[:, s, :])
                    nc.scalar.activation(
                        probs[:, g * 512:g * 512 + kw], ps[:, :kw],
                        mybir.ActivationFunctionType.Exp,
                        scale=SCALE, bias=(cbias[:, :] if fp8 else 0.0),
                        accum_out=lsum[:, g:g + 1])

            def emit_rest_a(s):
                """Transpose the slot's probability tiles into SBUF."""
                L = CAP[s]
                fp8 = PV_FP8 and s > 0
                pdt = f8e4 if fp8 else bf16
                probs, _ = slot_bufs[s]
                pts = []
                for kg in range((L + 3) // 4):     # 4 transposes per bank,
                    kn = min(4, L - kg * 4)        # one wide copy per group
                    tp = ps_tr.tile([P, 512], bf16, tag="tr", name=f"tp{s}")
                    for j in range(kn):
                        kt = kg * 4 + j
                        nc.tensor.transpose(tp[:, j * P:(j + 1) * P],
                                            probs[:, kt * P:(kt + 1) * P],
                                            ident[:, :])
                    pT = pt_pool.tile([P, 4, P], pdt, tag="pT",
                                      name=f"pT{s}_{kg}")
                    nc.vector.tensor_copy(pT[:, 0:kn, :], tp[:, 0:kn * P])
                    pts.append(pT)
                slot_bufs[s] = (probs, slot_bufs[s][1], pts)

            def emit_rest_b(s):
                """PV matmuls into Y, then Y -> SBUF bf16 halves."""
                L = CAP[s]
                nk = L * P
                nkg = (nk + 511) // 512
                fp8 = PV_FP8 and s > 0
                probs, lsum, pts = slot_bufs.pop(s)
                rinv = small.tile([P, 1], f32, tag="rinv", name=f"rinv{s}")
                if nkg > 1:
                    rsum = small.tile([P, 1], f32, tag="rsum",
                                      name=f"rsum{s}")
                    nc.vector.tensor_reduce(rsum[:, :], lsum[:, 0:nkg],
                                            axis=mybir.AxisListType.X,
                                            op=mybir.AluOpType.add)
                    nc.vector.reciprocal(rinv[:, :], rsum[:, :])
                else:
                    nc.vector.reciprocal(rinv[:, :], lsum[:, 0:1])

                out_ps = ps_o.tile([P, D], f32, tag="mmout",
                                   name=f"ops{s}")
                for kg in range((L + 3) // 4):
                    kn = min(4, L - kg * 4)
                    pT = pts[kg]
                    if fp8:
                        for j2 in range(kn // 2):
                            kt = kg * 4 + j2 * 2
                            xv = xv8_h[kt // (NT // 2)]
                            kb = kt % (NT // 2)
                            nc.tensor.matmul(
                                out_ps[:, 0:512],
                                pT[:, j2 * 2:j2 * 2 + 2, :],
                                xv[:, kb:kb + 2, 0:512],
                                start=(kt == 0), stop=(kt == L - 2),
                                perf_mode=DR)
                            nc.tensor.matmul(
                                out_ps[:, 512:D],
                                pT[:, j2 * 2:j2 * 2 + 2, :],
                                xv[:, kb:kb + 2, 512:D],
                                start=(kt == 0), stop=(kt == L - 2),
                                perf_mode=DR)
                    else:
                        for j in range(kn):
                            kt = kg * 4 + j
                            nc.tensor.matmul(out_ps[:, 0:512],
                                             pT[:, j, :],
                                             xv16_sb[:, kt, 0:512],
                                             start=(kt == 0),
                                             stop=(kt == L - 1))
                            nc.tensor.matmul(out_ps[:, 512:D],
                                             pT[:, j, :],
                                             xv16_sb[:, kt, 512:D],
                                             start=(kt == 0),
                                             stop=(kt == L - 1))
                # out_ps holds Y = probs @ x_kv [q, d]; stage to SBUF
                y_lo = osb_pool.tile([P, 512], bf16, tag="ylo", name=f"ylo{s}")
                y_hi = osb_pool.tile([P, 256], bf16, tag="yhi", name=f"yhi{s}")
                nc.scalar.copy(y_lo[:, :], out_ps[:, 0:512])
                nc.vector.tensor_copy(y_hi[:, :], out_ps[:, 512:D])
                slot_y[s] = (y_lo, y_hi, rinv)
                slot_y_ps[s] = out_ps

            def emit_rest(s):
                """Transpose Y and apply Wv: out = (Y @ Wv^T) / l."""
                vt8 = VT_FP8 and s > 0
                y_lo, y_hi, rinv = slot_y.pop(s)
                ydt = f8e4 if vt8 else bf16
                ytT = pt_pool.tile([P, DC, P], ydt, tag="ytT", name=f"ytT{s}")
                for kg in range(2):
                    kn = 4 if kg == 0 else 2
                    tp = ps_tr.tile([P, 512], bf16, tag="tr", name=f"ytp{s}")
                    for j in range(kn):
                        dt_ = kg * 4 + j
                        ysrc = (y_lo[:, dt_ * P:(dt_ + 1) * P] if dt_ < 4 else
                                y_hi[:, (dt_ - 4) * P:(dt_ - 3) * P])
                        nc.tensor.transpose(tp[:, j * P:(j + 1) * P],
                                            ysrc, ident[:, :])
                    nc.vector.tensor_copy(ytT[:, kg * 4:kg * 4 + kn, :],
                                          tp[:, 0:kn * P])
                # reuse the Y psum tile: its value was drained to SBUF by
                # the y_lo/y_hi copies, and start=True resets the bank
                out2_ps = slot_y_ps.pop(s)
                if vt8:
                    for c2 in range(DC // 2):
                        nc.tensor.matmul(out2_ps[:, 0:512],
                                         ytT[:, c2 * 2:c2 * 2 + 2, :],
                                         wv8_sb[:, c2 * 2:c2 * 2 + 2, 0:512],
                                         start=(c2 == 0),
                                         stop=(c2 == DC // 2 - 1),
                                         perf_mode=DR)
                        nc.tensor.matmul(out2_ps[:, 512:D],
                                         ytT[:, c2 * 2:c2 * 2 + 2, :],
                                         wv8_sb[:, c2 * 2:c2 * 2 + 2, 512:D],
                                         start=(c2 == 0),
                                         stop=(c2 == DC // 2 - 1),
                                         perf_mode=DR)
                else:
                    for dc in range(DC):
                        nc.tensor.matmul(out2_ps[:, 0:512],
                                         ytT[:, dc, :],
                                         wv16_c(dc)[:, 0:512],
                                         start=(dc == 0), stop=(dc == DC - 1))
                    for dc in range(DC):
                        nc.tensor.matmul(out2_ps[:, 512:D],
                                         ytT[:, dc, :],
                                         wv16_c(dc)[:, 512:D],
                                         start=(dc == 0), stop=(dc == DC - 1))
                out_sb = osb_pool.tile([P, D], f16, tag="osb", name=f"osb{s}")
                for hcol in range(2):   # halves: DMA overlaps the next scale
                    cs = slice(hcol * 384, (hcol + 1) * 384)
                    nc.vector.tensor_scalar_mul(out_sb[:, cs], out2_ps[:, cs],
                                                rinv[:, :])
                    nc.sync.dma_start(out=out_d[s * P:(s + 1) * P, cs],
                                      in_=out_sb[:, cs])

            # software pipeline: scores(s+2) and probs-transposes(s+1) fill
            # the PE while slot s's PV matmuls wait on pT vector copies, and
            # slot s-1's Y-transposes/Wv matmuls wait on slot s-1's Y copies
            emit_scores(0)
            emit_scores(1)
            emit_rest_a(0)
            for s in range(NSLOT):
                if s + 2 < NSLOT:
                    emit_scores(s + 2)
                if s + 1 < NSLOT:
                    emit_rest_a(s + 1)
                emit_rest_b(s)
                emit_rest(s)

    nc.compile()
    return nc


def _pack(matT, dtype=BF16):
    """[D, W] (transposed operand) -> [P, DC, W] chunk layout."""
    d, w = matT.shape
    return np.ascontiguousarray(
        matT.reshape(d // P, P, w).transpose(1, 0, 2)).astype(dtype)


def shard_inputs(x, Wq, Wk, Wv):
    x = np.asarray(x, dtype=np.float32)
    Wq = np.asarray(Wq, np.float32)
    Wk = np.asarray(Wk, np.float32)
    Wv = np.asarray(Wv, np.float32)
    M = Wq.T @ Wk                                        # [D_in, D_in]
    mT = _pack(M)                                        # [P, DC, D]
    mT = np.ascontiguousarray(                           # [P, 3, DC, 256]
        mT.reshape(P, DC, 3, 256).transpose(0, 2, 1, 3))
    wvT = _pack(Wv.T)                                    # [P, DC, D] bf16
    wv8 = wvT.astype(F8E4)
    in_maps = []
    for c in range(N_CORES):
        b, side = divmod(c, 2)
        qtiles = SIDE_A if side == 0 else SIDE_B
        xb = x[b]                                    # [S, D]
        xkvT = _pack(np.ascontiguousarray(xb.T))         # [P, DC, S]
        xkvT = np.ascontiguousarray(                     # [P, 2, DC, S//2]
            xkvT.reshape(P, DC, 2, S // 2).transpose(0, 2, 1, 3))
        xvR = np.ascontiguousarray(                      # [P, NT, D] row-major
            xb.reshape(NT, P, D).transpose(1, 0, 2))
        xv8 = xvR.astype(F8E4)
        xv16 = xvR.astype(BF16)
        xq = np.concatenate([xb[t * P:(t + 1) * P] for t in qtiles], axis=0)
        xqT = _pack(np.ascontiguousarray(xq.T))          # [P, DC, QROWS]
        xqT = np.ascontiguousarray(                      # [P, 2, DC, 512]
            xqT.reshape(P, DC, 2, 512).transpose(0, 2, 1, 3))
        mask = np.empty((NSLOT, P, 256), np.float32)  # cast bf16 below
        for s, t in enumerate(qtiles):
            L = CAP[s]
            qidx = t * P + np.arange(P)[:, None]
            kidx = (L - 2) * P + np.arange(256)[None, :]
            mask[s] = np.where(kidx <= qidx, 0.0, -1e30).astype(np.float32)
        # mask dram layout [P, NSLOT, 256]
        mask = np.ascontiguousarray(mask.transpose(1, 0, 2)).astype(BF16)
        in_maps.append({"xqT": xqT, "mT": mT, "xkvT": xkvT, "xv8": xv8,
                        "xv16": xv16, "wv8": wv8, "wv16": wvT, "mask": mask})
    return in_maps


def unshard(results):
    out = np.empty((B, S, D), np.float32)
    for c in range(N_CORES):
        b, side = divmod(c, 2)
        qtiles = SIDE_A if side == 0 else SIDE_B
        oc = np.asarray(results[c]["out"], dtype=np.float32)
        for s, t in enumerate(qtiles):
            out[b, t * P:(t + 1) * P] = oc[s * P:(s + 1) * P]
    return out


def run(inputs, trace=False, trace_cores=None):
    """Run on hardware; returns (output, BassKernelResults)."""
    global _NC
    if _NC is None:
        _NC = build()
    in_maps = shard_inputs(inputs["x"], inputs["Wq"], inputs["Wk"],
                           inputs["Wv"])
    res = run_bass_kernel_spmd(_NC, in_maps, core_ids=list(range(N_CORES)),
                               trace=trace, trace_cores=trace_cores)
    return unshard(res.results), res


def kernel(x, Wq, Wk, Wv):
    out, _ = run({"x": x, "Wq": Wq, "Wk": Wk, "Wv": Wv})
    return out


# revision 38
# speedup vs baseline: 1.0347x; 1.0347x over previous
"""Trainium2 Bass kernel for single-head causal attention.

Problem: x:[4,2048,768], Wq/Wk/Wv:[768,768] (torch-Linear layout, y = x @ W.T),
out = causal_softmax(q k^T / sqrt(768)) @ v, all float32.

Sharding (8 NeuronCores, no collectives):
  - core pair (2b, 2b+1) handles batch b.
  - per batch, the 16 query tiles of 128 rows are split between the pair as
    {0,3,4,7,8,11,12,15} and {1,2,5,6,9,10,13,14}. Sorted by causal length
    those are {1,4,5,8,9,12,13,16} and {2,3,6,7,10,11,14,15} key-tiles, so
    both sides fit the same static per-slot key budget {2,4,...,16}: the one
    SPMD graph processes 8 query tiles whose key ranges are padded by at most
    one 128-tile and the pad/diagonal is handled by a host-provided additive
    mask over the last two key blocks of every slot.

Math restructuring vs the naive graph:
  - scores = (x Wq^T)(x Wk^T)^T = x (Wq^T Wk) x^T.  The host precomputes
    M = Wq^T @ Wk once, the device projects only the query rows (z = x_q M)
    and uses raw x^T (already needed as a DMA input) as the key-side operand.
    This deletes the whole K projection, which was computed redundantly by
    both cores of a pair (the single biggest PE-time item).
  - out = (probs @ x) @ Wv^T (the "V-trick"): V is never projected for the
    full sequence; the per-query Y = probs @ x is projected instead.
  - value path (probs @ x and Y @ Wv^T) runs in fp8 e4m3 with DoubleRow
    perf mode (2 contraction tiles per instruction) for slots >= 1; slot 0
    (query tiles 0/1, few-key rows, where softmax weight noise does not
    average out) stays bf16.  exp is computed as exp(s/sqrt(d) - 3) on fp8
    slots so probs fit e4m3's +/-240 range; the shift cancels in the
    softmax normalization.
"""

import math
import os
import sys

import numpy as np

if not any(os.path.isdir(os.path.join(p, "concourse")) for p in sys.path):
    sys.path.insert(0, "/opt/trn_rl_repo")

import concourse.bass as bass  # noqa: E402
import concourse.mybir as mybir  # noqa: E402
from concourse import bacc, tile  # noqa: E402
from concourse.bass_utils import run_bass_kernel_spmd  # noqa: E402
from concourse.masks import make_identity  # noqa: E402

import ml_dtypes  # noqa: E402

B, S, D = 4, 2048, 768
P = 128
NT = S // P          # 16 key tiles per batch
DC = D // P          # 6 contraction chunks
NSLOT = 8            # query tiles per core
QROWS = NSLOT * P    # 1024 query rows per core
N_CORES = 8
SCALE = 1.0 / math.sqrt(D)
C_SHIFT = 3.0        # exp bias on fp8 slots (cancels in normalization)

SIDE_A = [0, 3, 4, 7, 8, 11, 12, 15]   # causal lengths 1,4,5,8,9,12,13,16
SIDE_B = [1, 2, 5, 6, 9, 10, 13, 14]   # causal lengths 2,3,6,7,10,11,14,15
CAP = [2, 4, 6, 8, 10, 12, 14, 16]     # static key tiles per slot (>= real)

BF16 = ml_dtypes.bfloat16
F8E4 = ml_dtypes.float8_e4m3

PV_FP8 = True        # probs @ x in fp8 DoubleRow for slots >= 1
VT_FP8 = False       # fp8 DR loses to bf16 here (tiny MMs, unhidden LDW)

_NC = None


def build():
    """Build + compile the single SPMD graph run by all 8 cores."""
    f32 = mybir.dt.float32
    bf16 = mybir.dt.bfloat16
    f8e4 = mybir.dt.float8e4
    DR = mybir.MatmulPerfMode.DoubleRow

    nc = bacc.Bacc("TRN2", target_bir_lowering=False, debug=False,
                   num_devices=N_CORES)

    # inputs come pre-packed as [P, chunk, width] (host layout transform)
    xq_d = nc.dram_tensor("xqT", [P, 2, DC, 512], bf16,
                          kind="ExternalInput").ap()
    m_d = nc.dram_tensor("mT", [P, 3, DC, 256], bf16,
                         kind="ExternalInput").ap()
    xkv_d = nc.dram_tensor("xkvT", [P, 2, DC, S // 2], bf16,
                           kind="ExternalInput").ap()
    xv8_d = nc.dram_tensor("xv8", [P, NT, D], f8e4, kind="ExternalInput").ap()
    xv16_d = nc.dram_tensor("xv16", [P, NT, D], bf16,
                            kind="ExternalInput").ap()
    wv8_d = nc.dram_tensor("wv8", [P, DC, D], f8e4, kind="ExternalInput").ap()
    wv16_d = nc.dram_tensor("wv16", [P, DC, D], bf16,
                            kind="ExternalInput").ap()
    mask_d = nc.dram_tensor("mask", [P, NSLOT, 256], bf16,
                            kind="ExternalInput").ap()
    f16 = mybir.dt.float16
    out_d = nc.dram_tensor("out", [QROWS, D], f16,
                           kind="ExternalOutput").ap()

    with tile.TileContext(nc) as tc:
        with (
            tc.tile_pool(name="const", bufs=1) as const,
            tc.tile_pool(name="probs", bufs=6) as probs_pool,
            tc.tile_pool(name="lsums", bufs=5) as lsum_pool,
            tc.tile_pool(name="pt", bufs=8) as pt_pool,
            tc.tile_pool(name="osb", bufs=2) as osb_pool,
            tc.tile_pool(name="small", bufs=2) as small,
            tc.tile_pool(name="ps_s", bufs=2, space="PSUM") as ps_s,
            tc.tile_pool(name="ps_tr", bufs=2, space="PSUM") as ps_tr,
            tc.tile_pool(name="ps_o", bufs=2, space="PSUM") as ps_o,
        ):
            HC = DC // 2
            m_p = [const.tile([P, DC, 256], bf16, tag=f"mp{i}",
                              name=f"mp{i}") for i in range(3)]
            xq_g = [const.tile([P, DC, 512], bf16, tag=f"xqg{g}",
                               name=f"xqg{g}") for g in range(2)]
            xkv_k = [const.tile([P, DC, S // 2], bf16, tag=f"xkvk{h}",
                                name=f"xkvk{h}") for h in range(2)]
            wv16_h = [const.tile([P, HC, D], bf16, tag=f"wv16h{h}",
                                 name=f"wv16h{h}") for h in range(2)]
            wv8_sb = (const.tile([P, DC, D], f8e4, tag="wv8",
                                 name="wv8_sb")
                      if VT_FP8 else None)
            xv8_h = [const.tile([P, NT // 2, D], f8e4, tag=f"xv8h{h}",
                                name=f"xv8h{h}") for h in range(2)]
            nv16 = 2 if PV_FP8 else NT
            xv16_sb = const.tile([P, nv16, D], bf16, tag="xv16")
            mask_sb = const.tile([P, NSLOT, 256], bf16, tag="mask")
            ident = const.tile([P, P], bf16, tag="ident")
            cbias = const.tile([P, 1], f32, tag="cbias")
            nc.gpsimd.memset(cbias[:, :], -C_SHIFT)
            zt_sb = const.tile([P, DC, QROWS], bf16, tag="zt")

            def wv16_c(dc):
                return wv16_h[dc // HC][:, dc % HC, :]

            # priority-ordered input DMAs, split into ~128KB pieces so the
            # DMA ring completes them in issue order at full aggregate
            # bandwidth (whole-tensor transfers run on parallel queues and
            # fair-share, which finishes the first-needed bytes last)
            for c in range(0, DC, 2):
                nc.sync.dma_start(out=m_p[0][:, c:c + 2, :],
                                  in_=m_d[:, 0, c:c + 2, :])
            for dc in range(DC):
                nc.sync.dma_start(out=xq_g[0][:, dc, :],
                                  in_=xq_d[:, 0, dc, :])
            for i in range(1, 3):
                for c in range(0, DC, 2):
                    nc.sync.dma_start(out=m_p[i][:, c:c + 2, :],
                                      in_=m_d[:, i, c:c + 2, :])
            for dc in range(DC):
                nc.sync.dma_start(out=xq_g[1][:, dc, :],
                                  in_=xq_d[:, 1, dc, :])

            # HAM warm-up: keep the PE busy while wave 1 streams in so the
            # real matmuls run at 2.4GHz from the start.  The dummy operand
            # needs only a gpsimd memset (make_identity's affine_select
            # would delay the first warm-up matmul).
            dummy = const.tile([P, P], bf16, tag="dummy")
            nc.gpsimd.memset(dummy[:, :], 0.0)
            warm = ps_tr.tile([P, P], f32, tag="tr", name="warm")
            for _ in range(42):
                nc.tensor.matmul(warm[:, :], dummy[:, :], dummy[:, :],
                                 start=True, stop=True)
            make_identity(nc, ident[:, :])

            # ---- zT[e,q] projection: z = x_q @ (Wq^T Wk), group-major
            def zproj_group(g):
                for oc in range(DC):
                    ps = ps_s.tile([P, 512], f32, tag="mm512",
                                   name=f"zps{g}_{oc}")
                    for dc in range(DC):
                        nc.tensor.matmul(
                            ps[:, :],
                            m_p[oc // 2][:, dc,
                                         (oc % 2) * P:(oc % 2 + 1) * P],
                            xq_g[g][:, dc, :],
                            start=(dc == 0), stop=(dc == DC - 1))
                    nc.scalar.copy(zt_sb[:, oc, g * 512:(g + 1) * 512],
                                   ps[:, :])

            for g in range(QROWS // 512):
                zproj_group(g)
                if g == 0:
                    # key-side operands for the first score slots; gate the
                    # later inputs behind Zproj progress so the DMA ring's
                    # fair-share doesn't starve the critical-path transfers
                    nc.sync.dma_start(out=xkv_k[0][:, :, :],
                                      in_=xkv_d[:, 0, :, :])
                    nc.sync.dma_start(out=mask_sb[:, :, :],
                                      in_=mask_d[:, :, :])
                    gate = zt_sb[:, 0, 0:1]
                    nc.vector.tensor_copy(xv16_sb[:, 0, 0:1], gate)
                    nc.sync.dma_start(out=xv16_sb[:, :, :],
                                      in_=xv16_d[:, 0:nv16, :])
                    for h in range(2):
                        nc.vector.tensor_copy(wv16_h[h][:, 0, 0:1], gate)
                        nc.sync.dma_start(out=wv16_h[h][:, :, :],
                                          in_=wv16_d[:, h * HC:(h + 1) * HC,
                                                     :])
                else:
                    gate = zt_sb[:, 5, 0:1]
                    nc.vector.tensor_copy(xv8_h[0][:, 0, 0:1], gate)
                    nc.sync.dma_start(out=xv8_h[0][:, :, :],
                                      in_=xv8_d[:, 0:NT // 2, :])
                    nc.vector.tensor_copy(xkv_k[1][:, 0, 0:1], gate)
                    nc.sync.dma_start(out=xkv_k[1][:, :, :],
                                      in_=xkv_d[:, 1, :, :])
                    nc.vector.tensor_copy(xv8_h[1][:, 0, 0:1], gate)
                    nc.sync.dma_start(out=xv8_h[1][:, :, :],
                                      in_=xv8_d[:, NT // 2:NT, :])
                    if VT_FP8:
                        nc.sync.dma_start(out=wv8_sb[:, :, :],
                                          in_=wv8_d[:, :, :])

            slot_bufs = {}
            slot_y = {}
            slot_y_ps = {}

            def emit_scores(s):
                L = CAP[s]
                nk = L * P
                nkg = (nk + 511) // 512
                fp8 = PV_FP8 and s > 0
                probs = probs_pool.tile([P, S], bf16, tag="probs",
                                        name=f"probs{s}")
                lsum = lsum_pool.tile([P, 4], f32, tag="lsum",
                                      name=f"lsum{s}")
                slot_bufs[s] = (probs, lsum)
                for g in range(nkg):
                    kw = min(512, nk - g * 512)
                    ps = ps_s.tile([P, 512], f32, tag="mm512",
                                   name=f"sps{s}_{g}")
                    xk = xkv_k[g // 2]
                    ko = (g % 2) * 512
                    for dc in range(DC):
                        nc.tensor.matmul(
                            ps[:, :kw],
                            zt_sb[:, dc, s * P:(s + 1) * P],
                            xk[:, dc, ko:ko + kw],
                            start=(dc == 0), stop=(dc == DC - 1))
                    mstart = (L - 2) * P   # masked region: last two blocks
                    if g * 512 <= mstart < g * 512 + kw:
                        off = mstart - g * 512
                        nc.vector.tensor_add(ps[:, off:off + 256],
                                             ps[:, off:off + 256],
                                             mask_sb[:, s, :])
                    nc.scalar.activation(
                        probs[:, g * 512:g * 512 + kw], ps[:, :kw],
                        mybir.ActivationFunctionType.Exp,
                        scale=SCALE, bias=(cbias[:, :] if fp8 else 0.0),
                        accum_out=lsum[:, g:g + 1])

            def emit_rest_a(s):
                """Transpose the slot's probability tiles into SBUF."""
                L = CAP[s]
                fp8 = PV_FP8 and s > 0
                pdt = f8e4 if fp8 else bf16
                probs, _ = slot_bufs[s]
                pts = []
                for kg in range((L + 3) // 4):     # 4 transposes per bank,
                    kn = min(4, L - kg * 4)        # one wide copy per group
                    tp = ps_tr.tile([P, 512], bf16, tag="tr", name=f"tp{s}")
                    for j in range(kn):
                        kt = kg * 4 + j
                        nc.tensor.transpose(tp[:, j * P:(j + 1) * P],
                                            probs[:, kt * P:(kt + 1) * P],
                                            ident[:, :])
                    pT = pt_pool.tile([P, 4, P], pdt, tag="pT",
                                      name=f"pT{s}_{kg}")
                    nc.vector.tensor_copy(pT[:, 0:kn, :], tp[:, 0:kn * P])
                    pts.append(pT)
                slot_bufs[s] = (probs, slot_bufs[s][1], pts)

            def emit_rest_b(s):
                """PV matmuls into Y, then Y -> SBUF bf16 halves."""
                L = CAP[s]
                nk = L * P
                nkg = (nk + 511) // 512
                fp8 = PV_FP8 and s > 0
                probs, lsum, pts = slot_bufs.pop(s)
                rinv = small.tile([P, 1], f32, tag="rinv", name=f"rinv{s}")
                if nkg > 1:
                    rsum = small.tile([P, 1], f32, tag="rsum",
                                      name=f"rsum{s}")
                    nc.vector.tensor_reduce(rsum[:, :], lsum[:, 0:nkg],
                                            axis=mybir.AxisListType.X,
                                            op=mybir.AluOpType.add)
                    nc.vector.reciprocal(rinv[:, :], rsum[:, :])
                else:
                    nc.vector.reciprocal(rinv[:, :], lsum[:, 0:1])

                out_ps = ps_o.tile([P, D], f32, tag="mmout",
                                   name=f"ops{s}")
                for kg in range((L + 3) // 4):
                    kn = min(4, L - kg * 4)
                    pT = pts[kg]
                    if fp8:
                        for j2 in range(kn // 2):
                            kt = kg * 4 + j2 * 2
                            xv = xv8_h[kt // (NT // 2)]
                            kb = kt % (NT // 2)
                            nc.tensor.matmul(
                                out_ps[:, 0:512],
                                pT[:, j2 * 2:j2 * 2 + 2, :],
                                xv[:, kb:kb + 2, 0:512],
                                start=(kt == 0), stop=(kt == L - 2),
                                perf_mode=DR)
                            nc.tensor.matmul(
                                out_ps[:, 512:D],
                                pT[:, j2 * 2:j2 * 2 + 2, :],
                                xv[:, kb:kb + 2, 512:D],
                                start=(kt == 0), stop=(kt == L - 2),
                                perf_mode=DR)
                    else:
                        for j in range(kn):
                            kt = kg * 4 + j
                            nc.tensor.matmul(out_ps[:, 0:512],
                                             pT[:, j, :],
                                             xv16_sb[:, kt, 0:512],
                                             start=(kt == 0),
                                             stop=(kt == L - 1))
                            nc.tensor.matmul(out_ps[:, 512:D],
                                             pT[:, j, :],
                                             xv16_sb[:, kt, 512:D],
                                             start=(kt == 0),
                                             stop=(kt == L - 1))
                # out_ps holds Y = probs @ x_kv [q, d]; stage to SBUF
                y_lo = osb_pool.tile([P, 512], bf16, tag="ylo", name=f"ylo{s}")
                y_hi = osb_pool.tile([P, 256], bf16, tag="yhi", name=f"yhi{s}")
                nc.scalar.copy(y_lo[:, :], out_ps[:, 0:512])
                nc.vector.tensor_copy(y_hi[:, :], out_ps[:, 512:D])
                slot_y[s] = (y_lo, y_hi, rinv)
                slot_y_ps[s] = out_ps

            def emit_rest(s):
                """Transpose Y and apply Wv: out = (Y @ Wv^T) / l."""
                vt8 = VT_FP8 and s > 0
                y_lo, y_hi, rinv = slot_y.pop(s)
                ydt = f8e4 if vt8 else bf16
                ytT = pt_pool.tile([P, DC, P], ydt, tag="ytT", name=f"ytT{s}")
                for kg in range(2):
                    kn = 4 if kg == 0 else 2
                    tp = ps_tr.tile([P, 512], bf16, tag="tr", name=f"ytp{s}")
                    for j in range(kn):
                        dt_ = kg * 4 + j
                        ysrc = (y_lo[:, dt_ * P:(dt_ + 1) * P] if dt_ < 4 else
                                y_hi[:, (dt_ - 4) * P:(dt_ - 3) * P])
                        nc.tensor.transpose(tp[:, j * P:(j + 1) * P],
                                            ysrc, ident[:, :])
                    nc.vector.tensor_copy(ytT[:, kg * 4:kg * 4 + kn, :],
                                          tp[:, 0:kn * P])
                # reuse the Y psum tile: its value was drained to SBUF by
                # the y_lo/y_hi copies, and start=True resets the bank
                out2_ps = slot_y_ps.pop(s)
                if vt8:
                    for c2 in range(DC // 2):
                        nc.tensor.matmul(out2_ps[:, 0:512],
                                         ytT[:, c2 * 2:c2 * 2 + 2, :],
                                         wv8_sb[:, c2 * 2:c2 * 2 + 2, 0:512],
                                         start=(c2 == 0),
                                         stop=(c2 == DC // 2 - 1),
                                         perf_mode=DR)
                        nc.tensor.matmul(out2_ps[:, 512:D],
                                         ytT[:, c2 * 2:c2 * 2 + 2, :],
                                         wv8_sb[:, c2 * 2:c2 * 2 + 2, 512:D],
                                         start=(c2 == 0),
                                         stop=(c2 == DC // 2 - 1),
                                         perf_mode=DR)
                else:
                    for dc in range(DC):
                        nc.tensor.matmul(out2_ps[:, 0:512],
                                         ytT[:, dc, :],
                                         wv16_c(dc)[:, 0:512],
                                         start=(dc == 0), stop=(dc == DC - 1))
                    for dc in range(DC):
                        nc.tensor.matmul(out2_ps[:, 512:D],
                                         ytT[:, dc, :],
                                         wv16_c(dc)[:, 512:D],
                                         start=(dc == 0), stop=(dc == DC - 1))
                out_sb = osb_pool.tile([P, D], f16, tag="osb", name=f"osb{s}")
                for hcol in range(2):   # halves: DMA overlaps the next scale
                    cs = slice(hcol * 384, (hcol + 1) * 384)
                    nc.vector.tensor_scalar_mul(out_sb[:, cs], out2_ps[:, cs],
                                                rinv[:, :])
                    nc.sync.dma_start(out=out_d[s * P:(s + 1) * P, cs],
                                      in_=out_sb[:, cs])

            # software pipeline: scores(s+2) and probs-transposes(s+1) fill
            # the PE while slot s's PV matmuls wait on pT vector copies, and
            # slot s-1's Y-transposes/Wv matmuls wait on slot s-1's Y copies
            emit_scores(0)
            emit_scores(1)
            emit_rest_a(0)
            for s in range(NSLOT):
                if s + 2 < NSLOT:
                    emit_scores(s + 2)
                if s + 1 < NSLOT:
                    emit_rest_a(s + 1)
                emit_rest_b(s)
                emit_rest(s)

    nc.compile()
    return nc


def _pack(matT, dtype=BF16):
    """[D, W] (transposed operand) -> [P, DC, W] chunk layout."""
    d, w = matT.shape
    return np.ascontiguousarray(
        matT.reshape(d // P, P, w).transpose(1, 0, 2)).astype(dtype)


def shard_inputs(x, Wq, Wk, Wv):
    x = np.asarray(x, dtype=np.float32)
    Wq = np.asarray(Wq, np.float32)
    Wk = np.asarray(Wk, np.float32)
    Wv = np.asarray(Wv, np.float32)
    M = Wq.T @ Wk                                        # [D_in, D_in]
    mT = _pack(M)                                        # [P, DC, D]
    mT = np.ascontiguousarray(                           # [P, 3, DC, 256]
        mT.reshape(P, DC, 3, 256).transpose(0, 2, 1, 3))
    wvT = _pack(Wv.T)                                    # [P, DC, D] bf16
    wv8 = wvT.astype(F8E4)
    in_maps = []
    for c in range(N_CORES):
        b, side = divmod(c, 2)
        qtiles = SIDE_A if side == 0 else SIDE_B
        xb = x[b]                                    # [S, D]
        xkvT = _pack(np.ascontiguousarray(xb.T))         # [P, DC, S]
        xkvT = np.ascontiguousarray(                     # [P, 2, DC, S//2]
            xkvT.reshape(P, DC, 2, S // 2).transpose(0, 2, 1, 3))
        xvR = np.ascontiguousarray(                      # [P, NT, D] row-major
            xb.reshape(NT, P, D).transpose(1, 0, 2))
        xv8 = xvR.astype(F8E4)
        xv16 = xvR.astype(BF16)
        xq = np.concatenate([xb[t * P:(t + 1) * P] for t in qtiles], axis=0)
        xqT = _pack(np.ascontiguousarray(xq.T))          # [P, DC, QROWS]
        xqT = np.ascontiguousarray(                      # [P, 2, DC, 512]
            xqT.reshape(P, DC, 2, 512).transpose(0, 2, 1, 3))
        mask = np.empty((NSLOT, P, 256), np.float32)  # cast bf16 below
        for s, t in enumerate(qtiles):
            L = CAP[s]
            qidx = t * P + np.arange(P)[:, None]
            kidx = (L - 2) * P + np.arange(256)[None, :]
            mask[s] = np.where(kidx <= qidx, 0.0, -1e30).astype(np.float32)
        # mask dram layout [P, NSLOT, 256]
        mask = np.ascontiguousarray(mask.transpose(1, 0, 2)).astype(BF16)
        in_maps.append({"xqT": xqT, "mT": mT, "xkvT": xkvT, "xv8": xv8,
                        "xv16": xv16, "wv8": wv8, "wv16": wvT, "mask": mask})
    return in_maps


def unshard(results):
    out = np.empty((B, S, D), np.float32)
    for c in range(N_CORES):
        b, side = divmod(c, 2)
        qtiles = SIDE_A if side == 0 else SIDE_B
        oc = np.asarray(results[c]["out"], dtype=np.float32)
        for s, t in enumerate(qtiles):
            out[b, t * P:(t + 1) * P] = oc[s * P:(s + 1) * P]
    return out


def run(inputs, trace=False, trace_cores=None):
    """Run on hardware; returns (output, BassKernelResults)."""
    global _NC
    if _NC is None:
        _NC = build()
    in_maps = shard_inputs(inputs["x"], inputs["Wq"], inputs["Wk"],
                           inputs["Wv"])
    res = run_bass_kernel_spmd(_NC, in_maps, core_ids=list(range(N_CORES)),
                               trace=trace, trace_cores=trace_cores)
    return unshard(res.results), res


def kernel(x, Wq, Wk, Wv):
    out, _ = run({"x": x, "Wq": Wq, "Wk": Wk, "Wv": Wv})
    return out


# revision 40
# speedup vs baseline: 1.0351x; 1.0004x over previous
"""Trainium2 Bass kernel for single-head causal attention.

Problem: x:[4,2048,768], Wq/Wk/Wv:[768,768] (torch-Linear layout, y = x @ W.T),
out = causal_softmax(q k^T / sqrt(768)) @ v, all float32.

Sharding (8 NeuronCores, no collectives):
  - core pair (2b, 2b+1) handles batch b.
  - per batch, the 16 query tiles of 128 rows are split between the pair as
    {0,3,4,7,8,11,12,15} and {1,2,5,6,9,10,13,14}. Sorted by causal length
    those are {1,4,5,8,9,12,13,16} and {2,3,6,7,10,11,14,15} key-tiles, so
    both sides fit the same static per-slot key budget {2,4,...,16}: the one
    SPMD graph processes 8 query tiles whose key ranges are padded by at most
    one 128-tile and the pad/diagonal is handled by a host-provided additive
    mask over the last two key blocks of every slot.

Math restructuring vs the naive graph:
  - scores = (x Wq^T)(x Wk^T)^T = x (Wq^T Wk) x^T.  The host precomputes
    M = Wq^T @ Wk once, the device projects only the query rows (z = x_q M)
    and uses raw x^T (already needed as a DMA input) as the key-side operand.
    This deletes the whole K projection, which was computed redundantly by
    both cores of a pair (the single biggest PE-time item).
  - out = (probs @ x) @ Wv^T (the "V-trick"): V is never projected for the
    full sequence; the per-query Y = probs @ x is projected instead.
  - value path (probs @ x and Y @ Wv^T) runs in fp8 e4m3 with DoubleRow
    perf mode (2 contraction tiles per instruction) for slots >= 1; slot 0
    (query tiles 0/1, few-key rows, where softmax weight noise does not
    average out) stays bf16.  exp is computed as exp(s/sqrt(d) - 3) on fp8
    slots so probs fit e4m3's +/-240 range; the shift cancels in the
    softmax normalization.
"""

import math
import os
import sys

import numpy as np

if not any(os.path.isdir(os.path.join(p, "concourse")) for p in sys.path):
    sys.path.insert(0, "/opt/trn_rl_repo")

import concourse.bass as bass  # noqa: E402
import concourse.mybir as mybir  # noqa: E402
from concourse import bacc, tile  # noqa: E402
from concourse.bass_utils import run_bass_kernel_spmd  # noqa: E402
from concourse.masks import make_identity  # noqa: E402

import ml_dtypes  # noqa: E402

B, S, D = 4, 2048, 768
P = 128
NT = S // P          # 16 key tiles per batch
DC = D // P          # 6 contraction chunks
NSLOT = 8            # query tiles per core
QROWS = NSLOT * P    # 1024 query rows per core
N_CORES = 8
SCALE = 1.0 / math.sqrt(D)
C_SHIFT = 3.0        # exp bias on fp8 slots (cancels in normalization)

SIDE_A = [0, 3, 4, 7, 8, 11, 12, 15]   # causal lengths 1,4,5,8,9,12,13,16
SIDE_B = [1, 2, 5, 6, 9, 10, 13, 14]   # causal lengths 2,3,6,7,10,11,14,15
CAP = [2, 4, 6, 8, 10, 12, 14, 16]     # static key tiles per slot (>= real)

BF16 = ml_dtypes.bfloat16
F8E4 = ml_dtypes.float8_e4m3

PV_FP8 = True        # probs @ x in fp8 DoubleRow for slots >= 1
VT_FP8 = False       # fp8 DR loses to bf16 here (tiny MMs, unhidden LDW)

_NC = None


def build():
    """Build + compile the single SPMD graph run by all 8 cores."""
    f32 = mybir.dt.float32
    bf16 = mybir.dt.bfloat16
    f8e4 = mybir.dt.float8e4
    DR = mybir.MatmulPerfMode.DoubleRow

    nc = bacc.Bacc("TRN2", target_bir_lowering=False, debug=False,
                   num_devices=N_CORES)

    # inputs come pre-packed as [P, chunk, width] (host layout transform)
    xq_d = nc.dram_tensor("xqT", [P, 2, DC, 512], bf16,
                          kind="ExternalInput").ap()
    m_d = nc.dram_tensor("mT", [P, 3, DC, 256], bf16,
                         kind="ExternalInput").ap()
    xkv_d = nc.dram_tensor("xkvT", [P, 2, DC, S // 2], bf16,
                           kind="ExternalInput").ap()
    xv8_d = nc.dram_tensor("xv8", [P, NT, D], f8e4, kind="ExternalInput").ap()
    xv16_d = nc.dram_tensor("xv16", [P, NT, D], bf16,
                            kind="ExternalInput").ap()
    wv8_d = nc.dram_tensor("wv8", [P, DC, D], f8e4, kind="ExternalInput").ap()
    wv16_d = nc.dram_tensor("wv16", [P, DC, D], bf16,
                            kind="ExternalInput").ap()
    mask_d = nc.dram_tensor("mask", [P, NSLOT, 256], bf16,
                            kind="ExternalInput").ap()
    f16 = mybir.dt.float16
    out_d = nc.dram_tensor("out", [QROWS, D], f16,
                           kind="ExternalOutput").ap()

    with tile.TileContext(nc) as tc:
        with (
            tc.tile_pool(name="const", bufs=1) as const,
            tc.tile_pool(name="probs", bufs=6) as probs_pool,
            tc.tile_pool(name="lsums", bufs=5) as lsum_pool,
            tc.tile_pool(name="pt", bufs=8) as pt_pool,
            tc.tile_pool(name="osb", bufs=2) as osb_pool,
            tc.tile_pool(name="small", bufs=2) as small,
            tc.tile_pool(name="ps_s", bufs=2, space="PSUM") as ps_s,
            tc.tile_pool(name="ps_tr", bufs=2, space="PSUM") as ps_tr,
            tc.tile_pool(name="ps_o", bufs=2, space="PSUM") as ps_o,
        ):
            HC = DC // 2
            m_p = [const.tile([P, DC, 256], bf16, tag=f"mp{i}",
                              name=f"mp{i}") for i in range(3)]
            xq_g = [const.tile([P, DC, 512], bf16, tag=f"xqg{g}",
                               name=f"xqg{g}") for g in range(2)]
            xkv_k = [const.tile([P, DC, S // 2], bf16, tag=f"xkvk{h}",
                                name=f"xkvk{h}") for h in range(2)]
            wv16_h = [const.tile([P, HC, D], bf16, tag=f"wv16h{h}",
                                 name=f"wv16h{h}") for h in range(2)]
            wv8_sb = (const.tile([P, DC, D], f8e4, tag="wv8",
                                 name="wv8_sb")
                      if VT_FP8 else None)
            xv8_h = [const.tile([P, NT // 2, D], f8e4, tag=f"xv8h{h}",
                                name=f"xv8h{h}") for h in range(2)]
            nv16 = 2 if PV_FP8 else NT
            xv16_sb = const.tile([P, nv16, D], bf16, tag="xv16")
            mask_sb = const.tile([P, NSLOT, 256], bf16, tag="mask")
            ident = const.tile([P, P], bf16, tag="ident")
            cbias = const.tile([P, 1], f32, tag="cbias")
            nc.gpsimd.memset(cbias[:, :], -C_SHIFT)
            zt_sb = const.tile([P, DC, QROWS], bf16, tag="zt")

            def wv16_c(dc):
                return wv16_h[dc // HC][:, dc % HC, :]

            # priority-ordered input DMAs, split into ~128KB pieces so the
            # DMA ring completes them in issue order at full aggregate
            # bandwidth (whole-tensor transfers run on parallel queues and
            # fair-share, which finishes the first-needed bytes last)
            for c in range(0, DC, 2):
                nc.sync.dma_start(out=m_p[0][:, c:c + 2, :],
                                  in_=m_d[:, 0, c:c + 2, :])
            for dc in range(DC):
                nc.sync.dma_start(out=xq_g[0][:, dc, :],
                                  in_=xq_d[:, 0, dc, :])
            for i in range(1, 3):
                for c in range(0, DC, 2):
                    nc.sync.dma_start(out=m_p[i][:, c:c + 2, :],
                                      in_=m_d[:, i, c:c + 2, :])
            for dc in range(DC):
                nc.sync.dma_start(out=xq_g[1][:, dc, :],
                                  in_=xq_d[:, 1, dc, :])

            # HAM warm-up: keep the PE busy while wave 1 streams in so the
            # real matmuls run at 2.4GHz from the start.  The dummy operand
            # needs only a gpsimd memset (make_identity's affine_select
            # would delay the first warm-up matmul).
            dummy = const.tile([P, P], bf16, tag="dummy")
            nc.gpsimd.memset(dummy[:, :], 0.0)
            warm = ps_tr.tile([P, P], f32, tag="tr", name="warm")
            for _ in range(42):
                nc.tensor.matmul(warm[:, :], dummy[:, :], dummy[:, :],
                                 start=True, stop=True)
            make_identity(nc, ident[:, :])

            # ---- zT[e,q] projection: z = x_q @ (Wq^T Wk), group-major
            def zproj_group(g):
                for oc in range(DC):
                    ps = ps_s.tile([P, 512], f32, tag="mm512",
                                   name=f"zps{g}_{oc}")
                    for dc in range(DC):
                        nc.tensor.matmul(
                            ps[:, :],
                            m_p[oc // 2][:, dc,
                                         (oc % 2) * P:(oc % 2 + 1) * P],
                            xq_g[g][:, dc, :],
                            start=(dc == 0), stop=(dc == DC - 1))
                    nc.scalar.copy(zt_sb[:, oc, g * 512:(g + 1) * 512],
                                   ps[:, :])

            for g in range(QROWS // 512):
                zproj_group(g)
                if g == 0:
                    # key-side operands for the first score slots; gate the
                    # later inputs behind Zproj progress so the DMA ring's
                    # fair-share doesn't starve the critical-path transfers
                    nc.sync.dma_start(out=xkv_k[0][:, :, :],
                                      in_=xkv_d[:, 0, :, :])
                    nc.sync.dma_start(out=mask_sb[:, :, :],
                                      in_=mask_d[:, :, :])
                    gate = zt_sb[:, 0, 0:1]
                    nc.vector.tensor_copy(xv16_sb[:, 0, 0:1], gate)
                    nc.sync.dma_start(out=xv16_sb[:, :, :],
                                      in_=xv16_d[:, 0:nv16, :])
                    for h in range(2):
                        nc.vector.tensor_copy(wv16_h[h][:, 0, 0:1], gate)
                        nc.sync.dma_start(out=wv16_h[h][:, :, :],
                                          in_=wv16_d[:, h * HC:(h + 1) * HC,
                                                     :])
                else:
                    gate = zt_sb[:, 5, 0:1]
                    nc.vector.tensor_copy(xv8_h[0][:, 0, 0:1], gate)
                    nc.sync.dma_start(out=xv8_h[0][:, :, :],
                                      in_=xv8_d[:, 0:NT // 2, :])
                    nc.vector.tensor_copy(xkv_k[1][:, 0, 0:1], gate)
                    nc.sync.dma_start(out=xkv_k[1][:, :, :],
                                      in_=xkv_d[:, 1, :, :])
                    nc.vector.tensor_copy(xv8_h[1][:, 0, 0:1], gate)
                    nc.sync.dma_start(out=xv8_h[1][:, :, :],
                                      in_=xv8_d[:, NT // 2:NT, :])
                    if VT_FP8:
                        nc.sync.dma_start(out=wv8_sb[:, :, :],
                                          in_=wv8_d[:, :, :])

            slot_bufs = {}
            slot_y = {}
            slot_y_ps = {}

            def emit_scores(s):
                L = CAP[s]
                nk = L * P
                nkg = (nk + 511) // 512
                fp8 = PV_FP8 and s > 0
                probs = probs_pool.tile([P, S], bf16, tag="probs",
                                        name=f"probs{s}")
                lsum = lsum_pool.tile([P, 4], f32, tag="lsum",
                                      name=f"lsum{s}")
                slot_bufs[s] = (probs, lsum)
                for g in range(nkg):
                    kw = min(512, nk - g * 512)
                    ps = ps_s.tile([P, 512], f32, tag="mm512",
                                   name=f"sps{s}_{g}")
                    xk = xkv_k[g // 2]
                    ko = (g % 2) * 512
                    for dc in range(DC):
                        nc.tensor.matmul(
                            ps[:, :kw],
                            zt_sb[:, dc, s * P:(s + 1) * P],
                            xk[:, dc, ko:ko + kw],
                            start=(dc == 0), stop=(dc == DC - 1))
                    mstart = (L - 2) * P   # masked region: last two blocks
                    if g * 512 <= mstart < g * 512 + kw:
                        off = mstart - g * 512
                        nc.vector.tensor_add(ps[:, off:off + 256],
                                             ps[:, off:off + 256],
                                             mask_sb[:, s, :])
                    nc.scalar.activation(
                        probs[:, g * 512:g * 512 + kw], ps[:, :kw],
                        mybir.ActivationFunctionType.Exp,
                        scale=SCALE, bias=(cbias[:, :] if fp8 else 0.0),
                        accum_out=lsum[:, g:g + 1])

            def emit_rest_a(s):
                """Transpose the slot's probability tiles into SBUF."""
                L = CAP[s]
                fp8 = PV_FP8 and s > 0
                pdt = f8e4 if fp8 else bf16
                probs, _ = slot_bufs[s]
                pts = []
                for kg in range((L + 3) // 4):     # 4 transposes per bank,
                    kn = min(4, L - kg * 4)        # one wide copy per group
                    tp = ps_tr.tile([P, 512], bf16, tag="tr", name=f"tp{s}")
                    for j in range(kn):
                        kt = kg * 4 + j
                        nc.tensor.transpose(tp[:, j * P:(j + 1) * P],
                                            probs[:, kt * P:(kt + 1) * P],
                                            ident[:, :])
                    pT = pt_pool.tile([P, 4, P], pdt, tag="pT",
                                      name=f"pT{s}_{kg}")
                    nc.vector.tensor_copy(pT[:, 0:kn, :], tp[:, 0:kn * P])
                    pts.append(pT)
                slot_bufs[s] = (probs, slot_bufs[s][1], pts)

            def emit_rest_b(s):
                """PV matmuls into Y, then Y -> SBUF bf16 halves."""
                L = CAP[s]
                nk = L * P
                nkg = (nk + 511) // 512
                fp8 = PV_FP8 and s > 0
                probs, lsum, pts = slot_bufs.pop(s)
                rinv = small.tile([P, 1], f32, tag="rinv", name=f"rinv{s}")
                if nkg > 1:
                    rsum = small.tile([P, 1], f32, tag="rsum",
                                      name=f"rsum{s}")
                    nc.vector.tensor_reduce(rsum[:, :], lsum[:, 0:nkg],
                                            axis=mybir.AxisListType.X,
                                            op=mybir.AluOpType.add)
                    nc.vector.reciprocal(rinv[:, :], rsum[:, :])
                else:
                    nc.vector.reciprocal(rinv[:, :], lsum[:, 0:1])

                out_ps = ps_o.tile([P, D], f32, tag="mmout",
                                   name=f"ops{s}")
                for kg in range((L + 3) // 4):
                    kn = min(4, L - kg * 4)
                    pT = pts[kg]
                    if fp8:
                        for j2 in range(kn // 2):
                            kt = kg * 4 + j2 * 2
                            xv = xv8_h[kt // (NT // 2)]
                            kb = kt % (NT // 2)
                            nc.tensor.matmul(
                                out_ps[:, 0:512],
                                pT[:, j2 * 2:j2 * 2 + 2, :],
                                xv[:, kb:kb + 2, 0:512],
                                start=(kt == 0), stop=(kt == L - 2),
                                perf_mode=DR)
                            nc.tensor.matmul(
                                out_ps[:, 512:D],
                                pT[:, j2 * 2:j2 * 2 + 2, :],
                                xv[:, kb:kb + 2, 512:D],
                                start=(kt == 0), stop=(kt == L - 2),
                                perf_mode=DR)
                    else:
                        for j in range(kn):
                            kt = kg * 4 + j
                            nc.tensor.matmul(out_ps[:, 0:512],
                                             pT[:, j, :],
                                             xv16_sb[:, kt, 0:512],
                                             start=(kt == 0),
                                             stop=(kt == L - 1))
                            nc.tensor.matmul(out_ps[:, 512:D],
                                             pT[:, j, :],
                                             xv16_sb[:, kt, 512:D],
                                             start=(kt == 0),
                                             stop=(kt == L - 1))
                # out_ps holds Y = probs @ x_kv [q, d]; stage to SBUF
                y_lo = osb_pool.tile([P, 512], bf16, tag="ylo", name=f"ylo{s}")
                y_hi = osb_pool.tile([P, 256], bf16, tag="yhi", name=f"yhi{s}")
                nc.scalar.copy(y_lo[:, :], out_ps[:, 0:512])
                nc.vector.tensor_copy(y_hi[:, :], out_ps[:, 512:D])
                slot_y[s] = (y_lo, y_hi, rinv)
                slot_y_ps[s] = out_ps

            def emit_rest(s):
                """Transpose Y and apply Wv: out = (Y @ Wv^T) / l."""
                vt8 = VT_FP8 and s > 0
                y_lo, y_hi, rinv = slot_y.pop(s)
                ydt = f8e4 if vt8 else bf16
                ytT = pt_pool.tile([P, DC, P], ydt, tag="ytT", name=f"ytT{s}")
                for kg in range(2):
                    kn = 4 if kg == 0 else 2
                    tp = ps_tr.tile([P, 512], bf16, tag="tr", name=f"ytp{s}")
                    for j in range(kn):
                        dt_ = kg * 4 + j
                        ysrc = (y_lo[:, dt_ * P:(dt_ + 1) * P] if dt_ < 4 else
                                y_hi[:, (dt_ - 4) * P:(dt_ - 3) * P])
                        nc.tensor.transpose(tp[:, j * P:(j + 1) * P],
                                            ysrc, ident[:, :])
                    nc.vector.tensor_copy(ytT[:, kg * 4:kg * 4 + kn, :],
                                          tp[:, 0:kn * P])
                # reuse the Y psum tile: its value was drained to SBUF by
                # the y_lo/y_hi copies, and start=True resets the bank
                out2_ps = slot_y_ps.pop(s)
                if vt8:
                    for c2 in range(DC // 2):
                        nc.tensor.matmul(out2_ps[:, 0:512],
                                         ytT[:, c2 * 2:c2 * 2 + 2, :],
                                         wv8_sb[:, c2 * 2:c2 * 2 + 2, 0:512],
                                         start=(c2 == 0),
                                         stop=(c2 == DC // 2 - 1),
                                         perf_mode=DR)
                        nc.tensor.matmul(out2_ps[:, 512:D],
                                         ytT[:, c2 * 2:c2 * 2 + 2, :],
                                         wv8_sb[:, c2 * 2:c2 * 2 + 2, 512:D],
                                         start=(c2 == 0),
                                         stop=(c2 == DC // 2 - 1),
                                         perf_mode=DR)
                else:
                    for dc in range(DC):
                        nc.tensor.matmul(out2_ps[:, 0:512],
                                         ytT[:, dc, :],
                                         wv16_c(dc)[:, 0:512],
                                         start=(dc == 0), stop=(dc == DC - 1))
                    for dc in range(DC):
                        nc.tensor.matmul(out2_ps[:, 512:D],
                                         ytT[:, dc, :],
                                         wv16_c(dc)[:, 512:D],
                                         start=(dc == 0), stop=(dc == DC - 1))
                out_sb = osb_pool.tile([P, D], f16, tag="osb", name=f"osb{s}")
                for hcol in range(2):   # halves: DMA overlaps the next scale
                    cs = slice(hcol * 384, (hcol + 1) * 384)
                    nc.vector.tensor_scalar_mul(out_sb[:, cs], out2_ps[:, cs],
                                                rinv[:, :])
                    nc.sync.dma_start(out=out_d[s * P:(s + 1) * P, cs],
                                      in_=out_sb[:, cs])

            # software pipeline: scores(s+2) and probs-transposes(s+1) fill
            # the PE while slot s's PV matmuls wait on pT vector copies, and
            # slot s-1's Y-transposes/Wv matmuls wait on slot s-1's Y copies
            emit_scores(0)
            emit_scores(1)
            emit_rest_a(0)
            for s in range(NSLOT):
                if s + 2 < NSLOT:
                    emit_scores(s + 2)
                if s + 1 < NSLOT:
                    emit_rest_a(s + 1)
                emit_rest_b(s)
                emit_rest(s)

    nc.compile()
    return nc


def _pack(matT, dtype=BF16):
    """[D, W] (transposed operand) -> [P, DC, W] chunk layout."""
    d, w = matT.shape
    return np.ascontiguousarray(
        matT.reshape(d // P, P, w).transpose(1, 0, 2)).astype(dtype)


def shard_inputs(x, Wq, Wk, Wv):
    x = np.asarray(x, dtype=np.float32)
    Wq = np.asarray(Wq, np.float32)
    Wk = np.asarray(Wk, np.float32)
    Wv = np.asarray(Wv, np.float32)
    M = Wq.T @ Wk                                        # [D_in, D_in]
    mT = _pack(M)                                        # [P, DC, D]
    mT = np.ascontiguousarray(                           # [P, 3, DC, 256]
        mT.reshape(P, DC, 3, 256).transpose(0, 2, 1, 3))
    wvT = _pack(Wv.T)                                    # [P, DC, D] bf16
    wv8 = wvT.astype(F8E4)
    in_maps = []
    for c in range(N_CORES):
        b, side = divmod(c, 2)
        qtiles = SIDE_A if side == 0 else SIDE_B
        xb = x[b]                                    # [S, D]
        xkvT = _pack(np.ascontiguousarray(xb.T))         # [P, DC, S]
        xkvT = np.ascontiguousarray(                     # [P, 2, DC, S//2]
            xkvT.reshape(P, DC, 2, S // 2).transpose(0, 2, 1, 3))
        xvR = np.ascontiguousarray(                      # [P, NT, D] row-major
            xb.reshape(NT, P, D).transpose(1, 0, 2))
        xv8 = xvR.astype(F8E4)
        xv16 = xvR.astype(BF16)
        xq = np.concatenate([xb[t * P:(t + 1) * P] for t in qtiles], axis=0)
        xqT = _pack(np.ascontiguousarray(xq.T))          # [P, DC, QROWS]
        xqT = np.ascontiguousarray(                      # [P, 2, DC, 512]
            xqT.reshape(P, DC, 2, 512).transpose(0, 2, 1, 3))
        mask = np.empty((NSLOT, P, 256), np.float32)  # cast bf16 below
        for s, t in enumerate(qtiles):
            L = CAP[s]
            qidx = t * P + np.arange(P)[:, None]
            kidx = (L - 2) * P + np.arange(256)[None, :]
            mask[s] = np.where(kidx <= qidx, 0.0, -1e30).astype(np.float32)
        # mask dram layout [P, NSLOT, 256]
        mask = np.ascontiguousarray(mask.transpose(1, 0, 2)).astype(BF16)
        in_maps.append({"xqT": xqT, "mT": mT, "xkvT": xkvT, "xv8": xv8,
                        "xv16": xv16, "wv8": wv8, "wv16": wvT, "mask": mask})
    return in_maps


def unshard(results):
    out = np.empty((B, S, D), np.float32)
    for c in range(N_CORES):
        b, side = divmod(c, 2)
        qtiles = SIDE_A if side == 0 else SIDE_B
        oc = np.asarray(results[c]["out"], dtype=np.float32)
        for s, t in enumerate(qtiles):
            out[b, t * P:(t + 1) * P] = oc[s * P:(s + 1) * P]
    return out


def run(inputs, trace=False, trace_cores=None):
    """Run on hardware; returns (output, BassKernelResults)."""
    global _NC
    if _NC is None:
        _NC = build()
    in_maps = shard_inputs(inputs["x"], inputs["Wq"], inputs["Wk"],
                           inputs["Wv"])
    res = run_bass_kernel_spmd(_NC, in_maps, core_ids=list(range(N_CORES)),
                               trace=trace, trace_cores=trace_cores)
    return unshard(res.results), res


def kernel(x, Wq, Wk, Wv):
    out, _ = run({"x": x, "Wq": Wq, "Wk": Wk, "Wv": Wv})
    return out
